# revision 1
# baseline (speedup 1.0000x reference)
"""Trainium2 Bass kernel for nn_Encoder_67138928771138 (CfC/LTC encoder).

Per time step: ncps mixed-memory LSTM cell (LATENT=512) followed by a
WiredCfCCell with 3 sequential sparse-masked CfC layers (inter/command/motor).
T=256 steps, B=128. Output = final (h, c), each (128, 512) f32.

Strategy (pure data parallel, 8 cores, B_local=16):
  - Fully transposed dataflow: features on SBUF partitions, batch (16) on the
    free dim.  All matmuls use weights as the stationary operand (lhsT) and
    activations [K<=128, 16] as the moving operand; PSUM accumulates fp32.
  - h features are sigma-permuted into 4 dense 128-chunks:
      sigma = [inter[0:128] | command[0:128] | motor[0:128] |
               inter[128:216](88) ++ command[128:143](15) ++ motor[128:153](25)]
    so the carry h is [128 part, 4*16 cols].  The three layer-c1 leftovers are
    produced in base-0 piece tiles and DMA'd into chunk 3 (engines cannot move
    data across partitions; DMA can).  CfC layers slice h chunks directly.
  - LSTM bias (+1.0 on the forget gate) rides an extra "ones" row of the
    per-step input column, so z accumulates entirely in PSUM.
  - CfC: ti = sigmoid(xc@(wb-wa).T + (bb-ba)) -> single fused weight wt;
    w1*mask / w2*mask premultiplied host-side (loop-invariant weight prep).
  - Pointwise on ACT (sigmoid/tanh share one table set) and DVE.

kernel(**inputs) takes FULL inputs, shards batch over 8 cores, runs via
run_bass_kernel_spmd, and reassembles full (h, c).
"""

import sys

sys.path.insert(0, "/opt/trn_rl_repo")

import numpy as np
import ml_dtypes
from contextlib import ExitStack

import concourse.bass as bass  # noqa: F401
import concourse.bacc as bacc
import concourse.mybir as mybir
import concourse.tile as tile

# ---------------- problem constants (hardcoded per spec) ----------------
B, T, NV = 128, 256, 8
IN_DIM = NV + 1            # x ++ dt = 9
H = 512
G4 = 4 * H                 # 2048
MOTOR, COMMAND, INTER = 153, 143, 216
NCORES = 8
BL = B // NCORES           # 16

OUT_L = [INTER, COMMAND, MOTOR]                            # 216 143 153
IN_L = [IN_DIM + INTER, INTER + COMMAND, COMMAND + MOTOR]  # 225 359 296
C1_L = [o - 128 for o in OUT_L]                            # 88 15 25
C1_OFF = [0, 88, 103]      # piece offsets inside sigma-chunk 3
# K-chunk row splits of each CfC layer's input xc (original xc order)
KSPLIT = [
    [(0, 9), (9, 137), (137, 225)],
    [(0, 128), (128, 216), (216, 344), (344, 359)],
    [(0, 128), (128, 143), (143, 271), (271, 296)],
]

# sigma permutation of the 512 h features (4 dense chunks)
SIGMA = np.r_[0:128, 216:344, 359:487, 128:216, 344:359, 487:512]

F32 = mybir.dt.float32
BF16 = mybir.dt.bfloat16
AF = mybir.ActivationFunctionType


def build_nc(dtype_mm=BF16, t_steps=T):
    """Build the per-core Bass/Tile program (identical on all cores)."""
    nc = bacc.Bacc("TRN2", target_bir_lowering=False, debug=False)

    np_mm = ml_dtypes.bfloat16 if dtype_mm == BF16 else np.float32

    xdt = nc.dram_tensor("xdt", [IN_DIM + 1, t_steps * BL], dtype_mm,
                         kind="ExternalInput")
    wit = nc.dram_tensor("wit", [IN_DIM + 1, G4], dtype_mm, kind="ExternalInput")
    wrt = nc.dram_tensor("wrt", [H, G4], dtype_mm, kind="ExternalInput")
    cfc_d = [
        nc.dram_tensor(f"cfc{l}", [IN_L[l], 3 * OUT_L[l]], dtype_mm,
                       kind="ExternalInput")
        for l in range(3)
    ]
    bt_d = [
        nc.dram_tensor(f"bt{l}", [128, 96], F32, kind="ExternalInput")
        for l in range(3)
    ]
    h_out = nc.dram_tensor("h_out", [128, 96], F32, kind="ExternalOutput")
    c_out = nc.dram_tensor("c_out", [128, 64], F32, kind="ExternalOutput")

    with ExitStack() as ctx:
        tc = ctx.enter_context(tile.TileContext(nc))
        const = ctx.enter_context(tc.tile_pool(name="const", bufs=1))
        state = ctx.enter_context(tc.tile_pool(name="state", bufs=2))
        work = ctx.enter_context(tc.tile_pool(name="work", bufs=3))
        psum = ctx.enter_context(tc.tile_pool(name="psum", bufs=2, space="PSUM"))

        # ---- load constants ----
        s_xdt = const.tile([IN_DIM + 1, t_steps * BL], dtype_mm, tag="xdt")
        nc.sync.dma_start(out=s_xdt, in_=xdt[:])
        s_wit = const.tile([IN_DIM + 1, G4], dtype_mm, tag="wit")
        nc.sync.dma_start(out=s_wit, in_=wit[:])
        s_wr = []
        for k in range(4):
            tl = const.tile([128, G4], dtype_mm, tag=f"wr{k}")
            nc.sync.dma_start(out=tl, in_=wrt[128 * k:128 * (k + 1), :])
            s_wr.append(tl)
        s_cfc = []
        for l in range(3):
            tiles = []
            for ki, (r0, r1) in enumerate(KSPLIT[l]):
                tl = const.tile([r1 - r0, 3 * OUT_L[l]], dtype_mm, tag=f"cfc{l}_{ki}")
                nc.sync.dma_start(out=tl, in_=cfc_d[l][r0:r1, :])
                tiles.append(tl)
            s_cfc.append(tiles)
        s_bt = []
        for l in range(3):
            tl = const.tile([128, 96], F32, tag=f"bt{l}")
            nc.sync.dma_start(out=tl, in_=bt_d[l][:])
            s_bt.append(tl)

        # ---- initial state ----
        h_prev = state.tile([128, 64], dtype_mm, tag="h")
        c_prev = state.tile([128, 64], F32, tag="c")
        nc.vector.memset(h_prev, 0.0)
        nc.vector.memset(c_prev, 0.0)

        h_fin = const.tile([128, 48], F32, tag="hfin")   # f32 c0 blocks, last step
        p_fin = [const.tile([C1_L[l], 16], F32, tag=f"pfin{l}", name=f"pfin{l}")
                 for l in range(3)]

        for t in range(t_steps):
            xcol = s_xdt[:, t * BL:(t + 1) * BL]          # [10, 16] incl ones row
            xcol9 = s_xdt[0:IN_DIM, t * BL:(t + 1) * BL]  # [9, 16] for CfC L0

            # ---------------- LSTM gates: zT, 16 dense M-tiles ----------------
            zp = psum.tile([128, 256], F32, tag="zp")
            for g in range(4):
                for j in range(4):
                    m = 4 * g + j
                    col0 = 128 * m
                    o = zp[:, 16 * m:16 * m + 16]
                    nc.tensor.matmul(o, s_wit[:, col0:col0 + 128], xcol,
                                     start=True, stop=False)
                    for k in range(4):
                        nc.tensor.matmul(
                            o, s_wr[k][:, col0:col0 + 128],
                            h_prev[:, 16 * k:16 * k + 16],
                            start=False, stop=(k == 3))

            # ---------------- LSTM pointwise ----------------
            # gate blocks in zp: i=[0:64), ig=[64:128), fg=[128:192), og=[192:256)
            sig = work.tile([128, 192], F32, tag="sig")
            nc.scalar.activation(sig, zp[:, 64:256], AF.Sigmoid)
            tai = work.tile([128, 64], F32, tag="tai")
            nc.scalar.activation(tai, zp[:, 0:64], AF.Tanh)
            tmp = work.tile([128, 64], F32, tag="tmp")
            nc.vector.tensor_mul(tmp, tai, sig[:, 0:64])        # tanh(i)*sig(ig)
            c_new = state.tile([128, 64], F32, tag="c")
            nc.vector.tensor_mul(c_new, c_prev, sig[:, 64:128])  # c*sig(fg+1)
            nc.vector.tensor_add(c_new, c_new, tmp)
            tcc = work.tile([128, 64], F32, tag="tcc")
            nc.scalar.activation(tcc, c_new, AF.Tanh)
            hl = work.tile([128, 64], dtype_mm, tag="hl")        # h_lstm
            nc.vector.tensor_mul(hl, tcc, sig[:, 128:192])       # tanh(c)*sig(og)
            # base-0 copies of the command/motor c1 slices of h_lstm
            hcmd1 = work.tile([15, 16], dtype_mm, tag="hcmd1")
            nc.sync.dma_start(out=hcmd1, in_=hl[88:103, 48:64])
            hmot1 = work.tile([25, 16], dtype_mm, tag="hmot1")
            nc.sync.dma_start(out=hmot1, in_=hl[103:128, 48:64])

            # ---------------- CfC layers ----------------
            last = t == t_steps - 1
            h_new = state.tile([128, 64], dtype_mm, tag="h")
            pieces = [work.tile([C1_L[l], 16], dtype_mm, tag=f"p{l}", name=f"p{l}")
                      for l in range(3)]
            rhs_per_layer = [
                [xcol9, hl[:, 0:16], hl[0:88, 48:64]],
                [h_new[:, 0:16], pieces[0], hl[:, 16:32], hcmd1],
                [h_new[:, 16:32], pieces[1], hl[:, 32:48], hmot1],
            ]
            for l in range(3):
                ol, c1 = OUT_L[l], C1_L[l]
                rhs_list = rhs_per_layer[l]
                cp = psum.tile([128, 96], F32, tag=f"cp{l}")
                nc.vector.memset(cp, 0.0)   # junk rows of c1 blocks are ACT-read
                nk = len(rhs_list)
                for tau in range(3):
                    for cc in (0, 1):
                        w = 128 if cc == 0 else c1
                        o = cp[0:w, 16 * (2 * tau + cc):16 * (2 * tau + cc) + 16]
                        for ki, rhs in enumerate(rhs_list):
                            lhs = s_cfc[l][ki][:, tau * ol + 128 * cc:
                                               tau * ol + 128 * cc + w]
                            nc.tensor.matmul(o, lhs, rhs,
                                             start=ki == 0, stop=ki == nk - 1)
                # blocks in cp: ff1c0 ff1c1 ff2c0 ff2c1 tic0 tic1 (16 cols each)
                zc = work.tile([128, 96], F32, tag=f"zc{l}")
                nc.vector.tensor_add(zc, cp, s_bt[l])
                th = work.tile([128, 64], F32, tag=f"th{l}")
                nc.scalar.activation(th, zc[:, 0:64], AF.Tanh)
                sg = work.tile([128, 32], F32, tag=f"sg{l}")
                nc.scalar.activation(sg, zc[:, 64:96], AF.Sigmoid)
                d = work.tile([128, 32], F32, tag=f"d{l}")
                nc.vector.tensor_sub(d, th[:, 32:64], th[:, 0:32])  # ff2-ff1
                e = work.tile([128, 32], F32, tag=f"e{l}")
                nc.vector.tensor_mul(e, sg, d)                      # ti*(ff2-ff1)
                # out = ff1 + ti*(ff2-ff1): c0 -> h_new chunk l, c1 -> piece tile
                nc.vector.tensor_add(h_new[:, 16 * l:16 * l + 16],
                                     th[:, 0:16], e[:, 0:16])
                nc.vector.tensor_add(pieces[l],
                                     th[0:c1, 16:32], e[0:c1, 16:32])
                if last:
                    nc.vector.tensor_add(h_fin[:, 16 * l:16 * l + 16],
                                         th[:, 0:16], e[:, 0:16])
                    nc.vector.tensor_add(p_fin[l],
                                         th[0:c1, 16:32], e[0:c1, 16:32])
            # assemble carry chunk 3 from the piece tiles (cross-partition: DMA)
            for l in range(3):
                nc.sync.dma_start(
                    out=h_new[C1_OFF[l]:C1_OFF[l] + C1_L[l], 48:64],
                    in_=pieces[l])

            h_prev, c_prev = h_new, c_new

        # ---- outputs (pre-zeroed; DMA only valid regions) ----
        nc.sync.dma_start(out=h_out[:, 0:48], in_=h_fin)
        for l in range(3):
            nc.sync.dma_start(out=h_out[0:C1_L[l], 48 + 16 * l:64 + 16 * l],
                              in_=p_fin[l])
        nc.sync.dma_start(out=c_out[:], in_=c_prev)

    nc.compile()
    return nc, np_mm


# ---------------- host-side input prep ----------------

def _prep_shared(inputs, np_mm):
    """Weight re-layout (pure per-parameter prep, no model compute)."""
    f = lambda a: np.asarray(a, np.float32)
    wi, wr, bi = f(inputs["lstm_wi"]), f(inputs["lstm_wr"]), f(inputs["lstm_bi"])
    bi_adj = bi.copy()
    bi_adj[2 * H:3 * H] += 1.0  # forget-gate +1
    row_perm = np.concatenate([g * H + SIGMA for g in range(4)])
    wi_p = wi[row_perm]
    bi_p = bi_adj[row_perm]
    wr_p = wr[np.ix_(row_perm, SIGMA)]
    wit = np.concatenate([wi_p, bi_p[:, None]], 1).T.astype(np_mm)  # [10, 2048]
    wrt = wr_p.T.astype(np_mm)                                      # [512, 2048]

    masks = [f(inputs["m0"]), f(inputs["m1"]), f(inputs["m2"])]
    cfc, bt = [], []
    for l in range(3):
        w1 = f(inputs[f"w1_{l}"]) * masks[l]
        w2 = f(inputs[f"w2_{l}"]) * masks[l]
        wt = f(inputs[f"wb_{l}"]) - f(inputs[f"wa_{l}"])
        cfc.append(np.concatenate([w1.T, w2.T, wt.T], 1).astype(np_mm))
        biases = [f(inputs[f"b1_{l}"]), f(inputs[f"b2_{l}"]),
                  f(inputs[f"bb_{l}"]) - f(inputs[f"ba_{l}"])]
        tile_b = np.zeros((128, 96), np.float32)
        ol = OUT_L[l]
        for tau in range(3):
            for cc in (0, 1):
                w = 128 if cc == 0 else ol - 128
                col = 16 * (2 * tau + cc)
                tile_b[0:w, col:col + 16] = biases[tau][128 * cc:128 * cc + w][:, None]
        bt.append(tile_b)
    return wit, wrt, cfc, bt


def _prep_xdt(inputs, core, np_mm, t_steps=T):
    x = np.asarray(inputs["x"], np.float32)[:, :t_steps]
    dt = np.asarray(inputs["dt"], np.float32)[:, :t_steps]
    b0 = core * BL
    xc = np.concatenate([x, dt], -1)[b0:b0 + BL]          # [16, T, 9]
    xc = xc.transpose(1, 2, 0)                            # [T, 9, 16]
    ones = np.ones((t_steps, 1, BL), np.float32)
    arr = np.concatenate([xc, ones], 1)                   # [T, 10, 16]
    return arr.transpose(1, 0, 2).reshape(IN_DIM + 1, t_steps * BL).astype(np_mm)


def _unpack_h(h_tile):
    """h_out [128, 96] -> [BL, 512] (undo sigma layout)."""
    res = np.zeros((BL, H), np.float32)
    hs = np.zeros((H, BL), np.float32)
    hs[0:128] = h_tile[:, 0:16]
    hs[128:256] = h_tile[:, 16:32]
    hs[256:384] = h_tile[:, 32:48]
    hs[384:472] = h_tile[0:88, 48:64]
    hs[472:487] = h_tile[0:15, 64:80]
    hs[487:512] = h_tile[0:25, 80:96]
    res[:, SIGMA] = hs.T
    return res


def _unpack_c(c_tile):
    """c_out [128, 64] (sigma chunks) -> [BL, 512]."""
    hs = np.concatenate([c_tile[:, 16 * k:16 * k + 16] for k in range(4)], 0)
    res = np.zeros((BL, H), np.float32)
    res[:, SIGMA] = hs.T
    return res


_CACHE = {}


def _get_nc(dtype_mm=BF16, t_steps=T):
    key = (dtype_mm, t_steps)
    if key not in _CACHE:
        _CACHE[key] = build_nc(dtype_mm, t_steps)
    return _CACHE[key]


def kernel(**inputs):
    from concourse.bass_utils import run_bass_kernel_spmd

    nc, np_mm = _get_nc()
    wit, wrt, cfc, bt = _prep_shared(inputs, np_mm)
    shared = {"wit": wit, "wrt": wrt,
              "cfc0": cfc[0], "cfc1": cfc[1], "cfc2": cfc[2],
              "bt0": bt[0], "bt1": bt[1], "bt2": bt[2]}
    in_maps = [dict(shared, xdt=_prep_xdt(inputs, c, np_mm)) for c in range(NCORES)]
    res = run_bass_kernel_spmd(nc, in_maps, core_ids=list(range(NCORES))).results
    h = np.concatenate([_unpack_h(res[c]["h_out"]) for c in range(NCORES)], 0)
    c = np.concatenate([_unpack_c(res[c]["c_out"]) for c in range(NCORES)], 0)
    return h, c



# revision 5
# speedup vs baseline: 1.0746x; 1.0746x over previous
"""Trainium2 Bass kernel for nn_Encoder_67138928771138 (CfC/LTC encoder).

Per time step: ncps mixed-memory LSTM cell (LATENT=512) followed by a
WiredCfCCell with 3 sequential sparse-masked CfC layers (inter/command/motor).
T=256 steps, B=128. Output = final (h, c), each (128, 512) f32.

Strategy (pure data parallel, 8 cores, B_local=16):
  - Fully transposed dataflow: features on SBUF partitions, batch (16) on the
    free dim.  All matmuls use weights as the stationary operand (lhsT) and
    activations [K<=128, 16] as the moving operand; PSUM accumulates fp32.
  - h features live in 4 dense sigma chunks:
      [inter[0:128] | command[0:128] | motor[0:128] |
       inter[128:216](88) ++ command[128:143](15) ++ motor[128:153](25)]
    The three CfC layer outputs are kept as separate [128, 32] tiles
    (cols 0:16 = c0 chunk, cols 16:32 rows 0:c1 = c1 piece); the LSTM
    recurrent weight wr is pre-split row-wise (wr0/wr1/wr2 for the c0
    chunks, wr3a/b/c for the pieces) so NO DMA is needed in the loop body.
  - CfC layer h-side c1 inputs are read from hl[:,48:64] with weight rows
    zero-padded to K=128 (no rebasing DMA).
  - Biases ride matmuls: LSTM bias (+1.0 forget) on the xdt ones row; CfC
    L0 bias on the K=10 x-chunk; CfC L1/L2 biases on a K=1 ones matmul that
    doubles as the PSUM block initializer (c1 blocks are zero-padded to
    M=128 so every PSUM row is written -> no memset).
  - PE work is issued in readiness order so next-step LSTM gate matmuls and
    h-dependent CfC matmuls overlap the serial ACT/DVE chain:
      [wit(t+1), L0 x-chunk, L1/L2 bias] -> (hl) [L0 h, L1 h, L2 h]
      -> (out0) [L1 in, wr0, wr3a] -> (out1) [L2 in, wr1, wr3b]
      -> (out2) [wr2, wr3c].

kernel(**inputs) takes FULL inputs, shards batch over 8 cores, runs via
run_bass_kernel_spmd, and reassembles full (h, c).
"""

import sys

sys.path.insert(0, "/opt/trn_rl_repo")

import numpy as np
import ml_dtypes
from contextlib import ExitStack

import concourse.bass as bass  # noqa: F401
import concourse.bacc as bacc
import concourse.mybir as mybir
import concourse.tile as tile

# ---------------- problem constants (hardcoded per spec) ----------------
B, T, NV = 128, 256, 8
IN_DIM = NV + 1            # x ++ dt = 9
H = 512
G4 = 4 * H                 # 2048
MOTOR, COMMAND, INTER = 153, 143, 216
NCORES = 8
BL = B // NCORES           # 16

OUT_L = [INTER, COMMAND, MOTOR]   # 216 143 153
C1_L = [o - 128 for o in OUT_L]   # 88 15 25
OFF_L = [0, INTER, INTER + COMMAND]  # feature offsets of layer outputs in h

# sigma permutation of the 512 h features (4 dense chunks; chunk3 = c1 pieces)
SIGMA = np.r_[0:128, 216:344, 359:487, 128:216, 344:359, 487:512]

F32 = mybir.dt.float32
BF16 = mybir.dt.bfloat16
AF = mybir.ActivationFunctionType

# per-layer lhsT chunk row counts (SBUF const tiles), in issue order
#   L0: k0 = x(9)+bias(1); kh0 = h c0 (128); kh1 = h c1 (88, base-0 rhs)
#   L1/L2: bias(1); kh0 = h c0 (128); kh1 = h c1 via hl[:,48:64] (128,
#          zero-padded); ki0 = prev out c0 (128); ki1 = prev out c1 (c1_prev)
L0_CHUNKS = [10, 128, 88]
L1_CHUNKS = [1, 128, 128, 128, 88]
L2_CHUNKS = [1, 128, 128, 128, 15]


def build_nc(dtype_mm=BF16, t_steps=T, bl=BL):
    """Build the per-core Bass/Tile program (identical on all cores)."""
    nc = bacc.Bacc("TRN2", target_bir_lowering=False, debug=False)

    np_mm = ml_dtypes.bfloat16 if dtype_mm == BF16 else np.float32

    xdt = nc.dram_tensor("xdt", [IN_DIM + 1, t_steps * bl], dtype_mm,
                         kind="ExternalInput")
    wit = nc.dram_tensor("wit", [IN_DIM + 1, G4], dtype_mm, kind="ExternalInput")
    wrt = nc.dram_tensor("wrt", [H, G4], dtype_mm, kind="ExternalInput")
    chunks_l = [L0_CHUNKS, L1_CHUNKS, L2_CHUNKS]
    cfc_d = [
        nc.dram_tensor(f"cfc{l}", [sum(chunks_l[l]), 768], dtype_mm,
                       kind="ExternalInput")
        for l in range(3)
    ]
    h_out = nc.dram_tensor("h_out", [128, 96], F32, kind="ExternalOutput")
    c_out = nc.dram_tensor("c_out", [128, 64], F32, kind="ExternalOutput")

    with ExitStack() as ctx:
        tc = ctx.enter_context(tile.TileContext(nc))
        const = ctx.enter_context(tc.tile_pool(name="const", bufs=1))
        state = ctx.enter_context(tc.tile_pool(name="state", bufs=2))
        work = ctx.enter_context(tc.tile_pool(name="work", bufs=2))
        psum_z = ctx.enter_context(tc.tile_pool(name="psum_z", bufs=2,
                                                space="PSUM"))
        psum_c = ctx.enter_context(tc.tile_pool(name="psum_c", bufs=1,
                                                space="PSUM"))

        # ---- load constants ----
        s_xdt = const.tile([IN_DIM + 1, t_steps * bl], dtype_mm, tag="xdt")
        nc.sync.dma_start(out=s_xdt, in_=xdt[:])
        s_wit = const.tile([IN_DIM + 1, G4], dtype_mm, tag="wit")
        nc.sync.dma_start(out=s_wit, in_=wit[:])
        # wr row-splits: c0 chunks (128 each) + c1 pieces (88/15/25)
        wr_rows = [(0, 128), (128, 256), (256, 384), (384, 472), (472, 487),
                   (487, 512)]
        s_wr = []
        for i, (r0, r1) in enumerate(wr_rows):
            tl = const.tile([r1 - r0, G4], dtype_mm, tag=f"wr{i}", name=f"wr{i}")
            nc.sync.dma_start(out=tl, in_=wrt[r0:r1, :])
            s_wr.append(tl)
        s_cfc = []
        for l in range(3):
            tiles, r0 = [], 0
            for ki, k in enumerate(chunks_l[l]):
                tl = const.tile([k, 768], dtype_mm, tag=f"cfc{l}_{ki}", name=f"cfc{l}_{ki}")
                nc.sync.dma_start(out=tl, in_=cfc_d[l][r0:r0 + k, :])
                tiles.append(tl)
                r0 += k
            s_cfc.append(tiles)
        s_one = const.tile([1, bl], dtype_mm, tag="one")
        nc.vector.memset(s_one, 1.0)

        h_fin = [const.tile([128, 32], F32, tag=f"hf{l}", name=f"hf{l}")
                 for l in range(3)]

        # ---- initial state: h = 0, c = 0 ----
        c_prev = state.tile([128, 64], F32, tag="c")
        nc.vector.memset(c_prev, 0.0)

        def zcols(t):
            return s_xdt[:, t * bl:(t + 1) * bl]

        # zp(0): h0 == 0, so only the wit contribution (incl. bias row)
        zp = psum_z.tile([128, 16 * bl], F32, tag="zp")
        x0 = zcols(0)
        for m in range(16):
            nc.tensor.matmul(zp[:, bl * m:bl * m + bl],
                             s_wit[:, 128 * m:128 * m + 128], x0,
                             start=True, stop=True)

        for t in range(t_steps):
            last = t == t_steps - 1
            # ---------------- LSTM pointwise on zp(t) ----------------
            # zp blocks: i=[0:4bl), ig=[4bl:8bl), fg=[8bl:12bl), og=[12bl:16bl)
            sig = work.tile([128, 12 * bl], F32, tag="sig")
            nc.scalar.activation(sig, zp[:, 4 * bl:16 * bl], AF.Sigmoid)
            tai = work.tile([128, 4 * bl], F32, tag="tai")
            nc.scalar.activation(tai, zp[:, 0:4 * bl], AF.Tanh)
            tmp = work.tile([128, 4 * bl], F32, tag="tmp")
            nc.vector.tensor_mul(tmp, tai, sig[:, 0:4 * bl])
            c_new = state.tile([128, 4 * bl], F32, tag="c")
            nc.vector.tensor_mul(c_new, c_prev, sig[:, 4 * bl:8 * bl])
            nc.vector.tensor_add(c_new, c_new, tmp)
            tcc = work.tile([128, 4 * bl], F32, tag="tcc")
            nc.scalar.activation(tcc, c_new, AF.Tanh)
            hl = work.tile([128, 4 * bl], dtype_mm, tag="hl")
            nc.vector.tensor_mul(hl, tcc, sig[:, 8 * bl:12 * bl])

            # NOTE: PSUM accumulation groups must be contiguous per region
            # (a start=True while another group in the same bank is open
            # resets it).  So each region's K-chunks are issued back-to-back
            # and the REGIONS are ordered by readiness of their last input;
            # the PE stalls inside a gated group while ACT/DVE run.
            cp = [psum_c.tile([128, 6 * bl], F32, tag=f"cp{l}", name=f"cp{l}")
                  for l in range(3)]
            hl3 = hl[:, 3 * bl:4 * bl]
            rhs_l = [
                [zcols(t), hl[:, 0:bl], hl[0:88, 3 * bl:4 * bl]],
                [s_one, hl[:, bl:2 * bl], hl3, None, None],   # ki* filled below
                [s_one, hl[:, 2 * bl:3 * bl], hl3, None, None],
            ]

            def cfc_layer_mm(l):
                rl = rhs_l[l]
                for b in range(6):
                    for ki, rhs in enumerate(rl):
                        nc.tensor.matmul(
                            cp[l][:, bl * b:bl * b + bl],
                            s_cfc[l][ki][:, 128 * b:128 * b + 128], rhs,
                            start=ki == 0, stop=ki == len(rl) - 1)

            cfc_layer_mm(0)

            # ---- CfC ACT/DVE chain; gated PE groups interleave ----
            outs = []
            for l in range(3):
                th = work.tile([128, 4 * bl], F32, tag=f"th{l}", name=f"th{l}")
                nc.scalar.activation(th, cp[l][:, 0:4 * bl], AF.Tanh)
                sg = work.tile([128, 2 * bl], F32, tag=f"sg{l}", name=f"sg{l}")
                nc.scalar.activation(sg, cp[l][:, 4 * bl:6 * bl], AF.Sigmoid)
                d = work.tile([128, 2 * bl], F32, tag=f"d{l}", name=f"d{l}")
                nc.vector.tensor_sub(d, th[:, 2 * bl:4 * bl], th[:, 0:2 * bl])
                e = work.tile([128, 2 * bl], F32, tag=f"e{l}", name=f"e{l}")
                nc.vector.tensor_mul(e, sg, d)
                out = state.tile([128, 2 * bl], dtype_mm, tag=f"o{l}", name=f"o{l}")
                nc.vector.tensor_add(out, th[:, 0:2 * bl], e)
                if last:
                    nc.vector.tensor_add(h_fin[l], th[:, 0:2 * bl], e)
                outs.append(out)
                if l < 2:
                    rhs_l[l + 1][3] = out[:, 0:bl]
                    rhs_l[l + 1][4] = out[0:C1_L[l], bl:2 * bl]
                    cfc_layer_mm(l + 1)

            # ---- zp(t+1): full per-region groups, gated on out2 at the end
            if not last:
                zp2 = psum_z.tile([128, 16 * bl], F32, tag="zp")
                oc = [(o[:, 0:bl], o[0:C1_L[i], bl:2 * bl])
                      for i, o in enumerate(outs)]
                zrhs = [zcols(t + 1), oc[0][0], oc[0][1], oc[1][0], oc[1][1],
                        oc[2][0], oc[2][1]]
                zlhs = [s_wit, s_wr[0], s_wr[3], s_wr[1], s_wr[4], s_wr[2],
                        s_wr[5]]
                for m in range(16):
                    for ki in range(7):
                        nc.tensor.matmul(
                            zp2[:, bl * m:bl * m + bl],
                            zlhs[ki][:, 128 * m:128 * m + 128], zrhs[ki],
                            start=ki == 0, stop=ki == 6)

            c_prev = c_new
            if not last:
                zp = zp2

        # ---- outputs ----
        for l in range(3):
            nc.sync.dma_start(out=h_out[:, 32 * l:32 * l + 32], in_=h_fin[l])
        nc.sync.dma_start(out=c_out[:], in_=c_prev)

    nc.compile()
    return nc, np_mm


# ---------------- host-side input prep ----------------

def _prep_shared(inputs, np_mm):
    """Weight re-layout (pure per-parameter prep, no model compute)."""
    f = lambda a: np.asarray(a, np.float32)
    wi, wr, bi = f(inputs["lstm_wi"]), f(inputs["lstm_wr"]), f(inputs["lstm_bi"])
    bi_adj = bi.copy()
    bi_adj[2 * H:3 * H] += 1.0  # forget-gate +1
    row_perm = np.concatenate([g * H + SIGMA for g in range(4)])
    wi_p = wi[row_perm]
    bi_p = bi_adj[row_perm]
    wr_p = wr[np.ix_(row_perm, SIGMA)]
    wit = np.concatenate([wi_p, bi_p[:, None]], 1).T.astype(np_mm)  # [10, 2048]
    wrt = wr_p.T.astype(np_mm)                                      # [512, 2048]

    masks = [f(inputs["m0"]), f(inputs["m1"]), f(inputs["m2"])]
    cfc = []
    for l in range(3):
        ol, c1 = OUT_L[l], C1_L[l]
        w1 = f(inputs[f"w1_{l}"]) * masks[l]
        w2 = f(inputs[f"w2_{l}"]) * masks[l]
        wt = f(inputs[f"wb_{l}"]) - f(inputs[f"wa_{l}"])
        b1, b2 = f(inputs[f"b1_{l}"]), f(inputs[f"b2_{l}"])
        bt = f(inputs[f"bb_{l}"]) - f(inputs[f"ba_{l}"])
        in_l = w1.shape[1]

        def blocks(rows):
            """[len(rows), 768]: W.T row-slices, out cols padded to 128/128."""
            out = np.zeros((len(rows), 768), np.float32)
            for bi_, w in enumerate((w1, w2, wt)):
                wt_ = w.T[rows]                       # [k, out_l]
                out[:, 256 * bi_:256 * bi_ + 128] = wt_[:, 0:128]
                out[:, 256 * bi_ + 128:256 * bi_ + 128 + c1] = wt_[:, 128:ol]
            return out

        def bias_row():
            out = np.zeros((1, 768), np.float32)
            for bi_, bv in enumerate((b1, b2, bt)):
                out[0, 256 * bi_:256 * bi_ + 128] = bv[0:128]
                out[0, 256 * bi_ + 128:256 * bi_ + 128 + c1] = bv[128:ol]
            return out

        if l == 0:
            # rows: x(9)+bias, h c0 (xc rows 9:137), h c1 (137:225)
            k0 = np.concatenate([blocks(np.arange(0, 9)), bias_row()], 0)
            kh0 = blocks(np.arange(9, 137))
            kh1 = blocks(np.arange(137, 225))
            arr = np.concatenate([k0, kh0, kh1], 0)
        else:
            n_in = OUT_L[l - 1]                       # prev layer out size
            h0r = np.arange(n_in, n_in + 128)         # h c0 xc rows
            h1r = np.arange(n_in + 128, in_l)         # h c1 xc rows
            kh0 = blocks(h0r)
            kh1 = np.zeros((128, 768), np.float32)
            r0 = 88 if l == 1 else 103                # rows inside hl[:,48:64]
            kh1[r0:r0 + len(h1r)] = blocks(h1r)
            ki0 = blocks(np.arange(0, 128))
            ki1 = blocks(np.arange(128, n_in))
            arr = np.concatenate([bias_row(), kh0, kh1, ki0, ki1], 0)
        cfc.append(arr.astype(np_mm))
    return wit, wrt, cfc


def _prep_xdt(inputs, core, np_mm, t_steps=T, bl=BL):
    x = np.asarray(inputs["x"], np.float32)[:, :t_steps]
    dt = np.asarray(inputs["dt"], np.float32)[:, :t_steps]
    b0 = core * bl
    xc = np.concatenate([x, dt], -1)[b0:b0 + bl]          # [bl, T, 9]
    xc = xc.transpose(1, 2, 0)                            # [T, 9, bl]
    ones = np.ones((t_steps, 1, bl), np.float32)
    arr = np.concatenate([xc, ones], 1)                   # [T, 10, bl]
    return arr.transpose(1, 0, 2).reshape(IN_DIM + 1, t_steps * bl).astype(np_mm)


def _shared_inputs(inputs, np_mm):
    wit, wrt, cfc = _prep_shared(inputs, np_mm)
    return {"wit": wit, "wrt": wrt,
            "cfc0": cfc[0], "cfc1": cfc[1], "cfc2": cfc[2]}


def _unpack_h(h_tile, bl=BL):
    """h_out [128, 6*bl] (3 layers x (c0|c1)) -> [bl, 512]."""
    res = np.zeros((bl, H), np.float32)
    for l in range(3):
        c0 = h_tile[:, 2 * bl * l:2 * bl * l + bl]
        c1 = h_tile[0:C1_L[l], 2 * bl * l + bl:2 * bl * l + 2 * bl]
        res[:, OFF_L[l]:OFF_L[l] + 128] = c0.T
        res[:, OFF_L[l] + 128:OFF_L[l] + OUT_L[l]] = c1.T
    return res


def _unpack_c(c_tile, bl=BL):
    """c_out [128, 4*bl] (sigma chunks) -> [bl, 512]."""
    hs = np.concatenate([c_tile[:, bl * k:bl * k + bl] for k in range(4)], 0)
    res = np.zeros((bl, H), np.float32)
    res[:, SIGMA] = hs.T
    return res


_CACHE = {}


def _get_nc(dtype_mm=BF16, t_steps=T):
    key = (dtype_mm, t_steps)
    if key not in _CACHE:
        _CACHE[key] = build_nc(dtype_mm, t_steps)
    return _CACHE[key]


def kernel(**inputs):
    from concourse.bass_utils import run_bass_kernel_spmd

    nc, np_mm = _get_nc()
    shared = _shared_inputs(inputs, np_mm)
    in_maps = [dict(shared, xdt=_prep_xdt(inputs, c, np_mm)) for c in range(NCORES)]
    res = run_bass_kernel_spmd(nc, in_maps, core_ids=list(range(NCORES))).results
    h = np.concatenate([_unpack_h(res[c]["h_out"]) for c in range(NCORES)], 0)
    c = np.concatenate([_unpack_c(res[c]["c_out"]) for c in range(NCORES)], 0)
    return h, c
